# revision 11
# baseline (speedup 1.0000x reference)
"""Trainium2 Bass kernel for nn_MultiHeadSelfAttention (B=8, S=1024, D=1024, H=16).

Sharding: data-parallel over batch — each of the 8 NeuronCores computes one
batch element end-to-end (projections, attention, softmax, output projection,
residual + layernorm) and returns its y[b] and attn[b] slices; the host stacks
them back into the full outputs.

Per-core dataflow (all matmuls in fp32r = full-rate fp32 on the PE):
  phase 1: Q^T, K^T = W @ x^T (dout on partitions), V = x @ W^T (seq on
           partitions). Biases bq/bk are added on the PSUM->SBUF copy;
           bv/bo are folded on the host (bo_eff = bo + Wo@bv into the
           residual input).
  phase 2 (per head pair, heads packed into array rows 0-63 / 64-127):
           sT = K^T-major scores -> exp -> E^T (PV operand),
           s  = Q-major scores -> exp (+row-sum Z) -> attn = E/Z -> DRAM,
           PV: ctx^T += V_h^T-slices @ E^T (2 heads packed into array
           columns via tile_position), normalized by 1/Z transposed via PE.
  phase 3: out = ctx @ Wo^T + residual, layernorm along features -> y.
"""

import numpy as np

import concourse.bass as bass
import concourse.mybir as mybir
import concourse.tile as tile
from concourse import bacc
from concourse.bass_utils import run_bass_kernel_spmd
from concourse.masks import make_identity

P = 128
S = 1024  # sequence length
D = 1024  # model dim
H = 16  # heads
DK = 64  # head dim
NT = D // P  # 8 tiles along any 1024 dim
PAIRS = H // 2
SCALE = 1.0 / np.sqrt(DK)
LN_EPS = 1e-5

F32 = mybir.dt.float32
F32R = mybir.dt.float32r
AF = mybir.ActivationFunctionType
ALU = mybir.AluOpType

PACK_PV = False


def _r(ap):
    return ap


def build(reps: int = 1):
    nc = bacc.Bacc("TRN2", target_bir_lowering=False, debug=False)

    xt = nc.dram_tensor("xt", [D, S], F32R, kind="ExternalInput").ap()
    wqt = nc.dram_tensor("wqt", [D, D], F32R, kind="ExternalInput").ap()
    wkt = nc.dram_tensor("wkt", [D, D], F32R, kind="ExternalInput").ap()
    wvt = nc.dram_tensor("wvt", [D, D], F32R, kind="ExternalInput").ap()
    wot = nc.dram_tensor("wot", [D, D], F32R, kind="ExternalInput").ap()
    bqp = nc.dram_tensor("bqp", [P, NT], F32, kind="ExternalInput").ap()
    bkp = nc.dram_tensor("bkp", [P, NT], F32, kind="ExternalInput").ap()
    xres = nc.dram_tensor("xres", [S, D], F32, kind="ExternalInput").ap()
    gamma = nc.dram_tensor("gamma", [P, D], F32, kind="ExternalInput").ap()
    beta = nc.dram_tensor("beta", [P, D], F32, kind="ExternalInput").ap()
    y = nc.dram_tensor("y", [S, D], F32, kind="ExternalOutput").ap()
    attn = nc.dram_tensor("attn", [H, S, S], F32, kind="ExternalOutput").ap()

    with tile.TileContext(nc) as tc:
        with (
            tc.tile_pool(name="consts", bufs=1) as cpool,
            tc.tile_pool(name="small", bufs=2) as spool,
            tc.tile_pool(name="u", bufs=1) as u,
            tc.tile_pool(name="qkv", bufs=1) as u2,
            tc.tile_pool(name="ps", bufs=1, space="PSUM") as ps,
        ):
            ident_f = cpool.tile([P, P], F32, tag="identf", name="ident_f")
            make_identity(nc, ident_f)
            ident = cpool.tile([P, P], F32R, tag="ident", name="ident")
            nc.vector.tensor_copy(ident[:], ident_f[:])
            bq_sb = cpool.tile([P, NT], F32, tag="bq", name="bq_sb")
            nc.sync.dma_start(bq_sb[:], bqp)
            bk_sb = cpool.tile([P, NT], F32, tag="bk", name="bk_sb")
            nc.sync.dma_start(bk_sb[:], bkp)
            gamma_sb = cpool.tile([P, D], F32, tag="gamma", name="gamma_sb")
            nc.sync.dma_start(gamma_sb[:], gamma)
            beta_sb = cpool.tile([P, D], F32, tag="beta", name="beta_sb")
            nc.sync.dma_start(beta_sb[:], beta)
            eps_sb = cpool.tile([P, 1], F32, tag="eps", name="eps_sb")
            nc.vector.memset(eps_sb[:], float(LN_EPS))
            ones_sb = cpool.tile([P, DK], F32, tag="ones", name="ones_sb")
            nc.vector.memset(ones_sb[:], 1.0)

            def body():
                # ---------- phase 1: projections ----------
                xt_sb = []
                for kt in range(NT):
                    t = u.tile([P, S], F32R, tag="xe", bufs=13, name=f"xt{kt}")
                    nc.sync.dma_start(t[:], xt[kt * P : (kt + 1) * P, :])
                    xt_sb.append(t)

                qt_sb, kt_sb, v_sb = [], [], []
                for proj, (wdram, dst, bias) in enumerate(
                    [(wqt, qt_sb, bq_sb), (wkt, kt_sb, bk_sb), (wvt, v_sb, None)]
                ):
                    w_sb = []
                    for kt in range(NT):
                        t = u.tile([P, D], F32R, tag="w", bufs=8, name=f"w{proj}_{kt}")
                        nc.sync.dma_start(t[:], wdram[kt * P : (kt + 1) * P, :])
                        w_sb.append(t)
                    for mt in range(NT):
                        psm = ps.tile([P, S], F32, tag="mm", bufs=2, name=f"pp{proj}_{mt}")
                        for half in range(2):
                            hs = slice(half * 512, half * 512 + 512)
                            for kt in range(NT):
                                if proj < 2:
                                    # out = W @ x^T tile: lhsT = W^T k-slice,
                                    # rhs = x^T k-tile
                                    lhsT = w_sb[kt][:, mt * P : (mt + 1) * P]
                                    rhs = xt_sb[kt][:, hs]
                                else:
                                    # V = x @ Wv^T: lhsT = x^T k-tile m-slice,
                                    # rhs = Wv^T k-tile
                                    lhsT = xt_sb[kt][:, mt * P : (mt + 1) * P]
                                    rhs = w_sb[kt][:, hs]
                                nc.tensor.matmul(
                                    psm[:, hs],
                                    _r(lhsT),
                                    _r(rhs),
                                    start=(kt == 0),
                                    stop=(kt == NT - 1),
                                )
                        out_t = u2.tile([P, S], F32R, tag="qkv", bufs=24, name=f"p{proj}_{mt}")
                        if bias is not None:
                            nc.vector.tensor_scalar_add(
                                out_t[:], psm[:], bias[:, mt : mt + 1]
                            )
                        else:
                            nc.vector.tensor_copy(out_t[:], psm[:])
                        dst.append(out_t)

                # ---------- phase 2: attention per head pair ----------
                ctx_sb = []
                for p in range(PAIRS):
                    qts, kts = qt_sb[p], kt_sb[p]
                    rowA, rowB = slice(0, DK), slice(DK, P)

                    # transposed scores -> E^T
                    et = {0: [], 1: []}
                    for kt in range(NT):
                        ks = slice(kt * P, (kt + 1) * P)
                        for hh, rows in ((0, rowA), (1, rowB)):
                            psm = ps.tile([P, S], F32, tag="mm", bufs=2, name=f"st{p}_{kt}_{hh}")
                            for half in range(2):
                                hs = slice(half * 512, half * 512 + 512)
                                nc.tensor.matmul(
                                    psm[:, hs],
                                    _r(kts[rows, ks]),
                                    _r(qts[rows, hs]),
                                    start=True,
                                    stop=True,
                                )
                            e = u.tile([P, S], F32R, tag="xe", bufs=13, name=f"et{p}_{kt}_{hh}")
                            nc.scalar.activation(e[:], psm[:], AF.Exp, scale=float(SCALE))
                            et[hh].append(e)

                    # forward scores -> E, Z, attn out
                    z_t = spool.tile([P, 2 * NT], F32, tag="z", bufs=2, name=f"z{p}")
                    r_t = spool.tile([P, 2 * NT], F32, tag="r", bufs=2, name=f"r{p}")
                    for qt in range(NT):
                        qs = slice(qt * P, (qt + 1) * P)
                        for hh, rows in ((0, rowA), (1, rowB)):
                            psm = ps.tile([P, S], F32, tag="mm", bufs=2, name=f"s{p}_{qt}_{hh}")
                            for half in range(2):
                                hs = slice(half * 512, half * 512 + 512)
                                nc.tensor.matmul(
                                    psm[:, hs],
                                    _r(qts[rows, qs]),
                                    _r(kts[rows, hs]),
                                    start=True,
                                    stop=True,
                                )
                            zc = qt + NT * hh
                            e = u.tile([P, S], F32, tag="e", bufs=3, name=f"e{p}_{qt}_{hh}")
                            nc.scalar.activation(
                                e[:],
                                psm[:],
                                AF.Exp,
                                scale=float(SCALE),
                                accum_out=z_t[:, zc : zc + 1],
                            )
                            nc.vector.reciprocal(
                                r_t[:, zc : zc + 1], z_t[:, zc : zc + 1]
                            )
                            nc.vector.tensor_scalar_mul(
                                e[:], e[:], r_t[:, zc : zc + 1]
                            )
                            nc.sync.dma_start(attn[2 * p + hh, qs, :], e[:])

                    # PV: ctx^T (unnormalized) accumulated over k tiles
                    vcol = slice(p * P, p * P + DK)
                    vcol2 = slice(p * P + DK, (p + 1) * P)
                    if PACK_PV:
                        pv = ps.tile([P, S], F32, tag="pv", bufs=1, name=f"pv{p}")
                        for kt in range(NT):
                            for half in range(2):
                                hs = slice(half * 512, half * 512 + 512)
                                nc.tensor.matmul(
                                    pv[0:DK, hs],
                                    _r(v_sb[kt][:, vcol]),
                                    _r(et[0][kt][:, hs]),
                                    start=(kt == 0),
                                    stop=(kt == NT - 1),
                                    tile_position=(0, 0),
                                )
                                nc.tensor.matmul(
                                    pv[DK:P, hs],
                                    _r(v_sb[kt][:, vcol2]),
                                    _r(et[1][kt][:, hs]),
                                    start=(kt == 0),
                                    stop=(kt == NT - 1),
                                    tile_position=(0, DK),
                                )
                        pvA, pvB = pv[0:DK, :], pv[DK:P, :]
                    else:
                        pva = ps.tile([DK, S], F32, tag="pv", bufs=1, name=f"pva{p}")
                        pvb = ps.tile([DK, S], F32, tag="pv2", bufs=1, name=f"pvb{p}")
                        for kt in range(NT):
                            for half in range(2):
                                hs = slice(half * 512, half * 512 + 512)
                                nc.tensor.matmul(
                                    pva[:, hs],
                                    _r(v_sb[kt][:, vcol]),
                                    _r(et[0][kt][:, hs]),
                                    start=(kt == 0),
                                    stop=(kt == NT - 1),
                                )
                                nc.tensor.matmul(
                                    pvb[:, hs],
                                    _r(v_sb[kt][:, vcol2]),
                                    _r(et[1][kt][:, hs]),
                                    start=(kt == 0),
                                    stop=(kt == NT - 1),
                                )
                        pvA, pvB = pva[:], pvb[:]

                    # normalize ctx^T by 1/Z: spread each R column across 64
                    # partitions via PE (R_rep = R-col replicated 64 wide;
                    # R_rep.T @ I lands [64, 128] in PSUM), then multiply.
                    ctx = u2.tile([P, S], F32R, tag="qkv", bufs=24, name=f"ctx{p}")
                    ctx_sb.append(ctx)
                    for qt in range(NT):
                        qs = slice(qt * P, (qt + 1) * P)
                        for hh, (rows, pvx) in enumerate(((rowA, pvA), (rowB, pvB))):
                            zc = qt + NT * hh
                            rrep = spool.tile(
                                [P, DK], F32R, tag="rrep", bufs=3, name=f"rr{p}_{qt}_{hh}"
                            )
                            nc.vector.tensor_scalar_mul(
                                rrep[:], ones_sb[:], r_t[:, zc : zc + 1]
                            )
                            rtb = ps.tile([DK, P], F32, tag="mm", bufs=2, name=f"rtb{p}_{qt}_{hh}")
                            nc.tensor.matmul(
                                rtb[:], _r(rrep[:]), _r(ident[:]), start=True, stop=True
                            )
                            rtb_sb = spool.tile(
                                [DK, P], F32, tag="rtbs", bufs=3, name=f"rb{p}_{qt}_{hh}"
                            )
                            nc.vector.tensor_copy(rtb_sb[:], rtb[:])
                            nc.vector.tensor_tensor(
                                ctx[rows, qs], pvx[:, qs], rtb_sb[:], ALU.mult
                            )

                # ---------- phase 3: output projection + residual + LN ----------
                wo_sb = []
                for ft in range(NT):
                    t = u.tile([P, D], F32R, tag="xe", bufs=13, name=f"wo{ft}")
                    nc.sync.dma_start(t[:], wot[ft * P : (ft + 1) * P, :])
                    wo_sb.append(t)

                for qt in range(NT):
                    qs = slice(qt * P, (qt + 1) * P)
                    xr = u.tile([P, D], F32, tag="w", bufs=8, name=f"xr{qt}")
                    nc.sync.dma_start(xr[:], xres[qs, :])
                    psm = ps.tile([P, D], F32, tag="mm", bufs=2, name=f"po{qt}")
                    for half in range(2):
                        hs = slice(half * 512, half * 512 + 512)
                        for ft in range(NT):
                            nc.tensor.matmul(
                                psm[:, hs],
                                _r(ctx_sb[ft][:, qs]),
                                _r(wo_sb[ft][:, hs]),
                                start=(ft == 0),
                                stop=(ft == NT - 1),
                            )
                    h_t = u.tile([P, D], F32, tag="e", bufs=3, name=f"h{qt}")
                    nc.vector.tensor_tensor(h_t[:], psm[:], xr[:], ALU.add)

                    stats = spool.tile([P, 2, 6], F32, tag="stats", bufs=2, name=f"st{qt}")
                    nc.vector.bn_stats(stats[:, 0, :], h_t[:, 0:512])
                    nc.vector.bn_stats(stats[:, 1, :], h_t[:, 512:1024])
                    mv = spool.tile([P, 2], F32, tag="mv", bufs=2, name=f"mv{qt}")
                    nc.vector.bn_aggr(mv[:], stats[:])
                    sd = spool.tile([P, 1], F32, tag="sd", bufs=2, name=f"sd{qt}")
                    nc.scalar.activation(
                        sd[:], mv[:, 1:2], AF.Sqrt, bias=eps_sb[:]
                    )
                    rstd = spool.tile([P, 1], F32, tag="rstd", bufs=2, name=f"rs{qt}")
                    nc.vector.reciprocal(rstd[:], sd[:])

                    y_t = u.tile([P, D], F32, tag="e", bufs=3, name=f"y{qt}")
                    nc.vector.tensor_scalar(
                        y_t[:],
                        h_t[:],
                        mv[:, 0:1],
                        rstd[:],
                        ALU.subtract,
                        ALU.mult,
                    )
                    nc.vector.tensor_tensor(y_t[:], y_t[:], gamma_sb[:], ALU.mult)
                    nc.vector.tensor_tensor(y_t[:], y_t[:], beta_sb[:], ALU.add)
                    nc.sync.dma_start(y[qs, :], y_t[:])

            if reps == 1:
                body()
            else:
                with tc.For_i(0, reps, 1):
                    body()

    nc.compile()
    return nc


_CACHE: dict = {}


def _get_nc(reps: int = 1):
    if reps not in _CACHE:
        _CACHE[reps] = build(reps)
    return _CACHE[reps]


def _prep_inputs(x, Wq, bq, Wk, bk, Wv, bv, Wo, bo, gamma, beta):
    f = lambda a: np.ascontiguousarray(np.asarray(a, dtype=np.float32))
    x = f(x)
    Wq, bq, Wk, bk = f(Wq), f(bq), f(Wk), f(bk)
    Wv, bv, Wo, bo = f(Wv), f(bv), f(Wo), f(bo)
    gamma, beta = f(gamma), f(beta)

    shared = {
        "wqt": np.ascontiguousarray(Wq.T),
        "wkt": np.ascontiguousarray(Wk.T),
        "wvt": np.ascontiguousarray(Wv.T),
        "wot": np.ascontiguousarray(Wo.T),
        "bqp": np.ascontiguousarray(bq.reshape(NT, P).T),
        "bkp": np.ascontiguousarray(bk.reshape(NT, P).T),
        "gamma": np.ascontiguousarray(np.broadcast_to(gamma, (P, D))),
        "beta": np.ascontiguousarray(np.broadcast_to(beta, (P, D))),
    }
    bo_eff = bo + Wo @ bv
    in_maps = []
    for b in range(x.shape[0]):
        in_maps.append(
            {
                "xt": np.ascontiguousarray(x[b].T),
                "xres": x[b] + bo_eff[None, :],
                **shared,
            }
        )
    return in_maps


def kernel(x, Wq, bq, Wk, bk, Wv, bv, Wo, bo, gamma, beta):
    in_maps = _prep_inputs(x, Wq, bq, Wk, bk, Wv, bv, Wo, bo, gamma, beta)
    nc = _get_nc()
    res = run_bass_kernel_spmd(nc, in_maps, core_ids=list(range(len(in_maps))))
    y = np.stack([r["y"] for r in res.results])
    attn = np.stack([r["attn"] for r in res.results])
    return y, attn
